# revision 74
# baseline (speedup 1.0000x reference)
"""Trainium2 Bass kernel for a custom LSTM cell.

Transposed layout (hidden on partitions, batch free), data-parallel over
batch across 8 cores. Gate biases ride the ACT engine's per-partition bias;
the boundary term is host-precomputed and DVE-added into the f-gate PSUM.

Precision: the g gate feeds tanh (derivative up to 1.0) and amplifies
quantization noise ~4x more than the sigmoid gates, so fp8 DoubleRow (2x PE
rate) is allocated per-gate: i/f/o take their ENTIRE h_prev contraction
(1024 rows) in fp8, g stays fully bf16. Measured end-to-end rel err
1.307e-2 vs the 2e-2 budget — lower than a uniform fp8 split, with 50%
more of the stream at 2x rate.
"""

import sys

sys.path.insert(0, "/opt/trn_rl_repo")

import numpy as np
import ml_dtypes

BF16 = ml_dtypes.bfloat16
FP8 = ml_dtypes.float8_e4m3  # matches mybir.dt.float8e4

B, IN, H = 8192, 512, 1024
NCORES = 8
BLOC = B // NCORES  # 1024 batch rows per core
KTOT = IN + H  # 1536 contraction
KT = KTOT // 12 * 12 // 128  # 12 k-tiles
KX = IN // 128  # 4 x k-tiles (bf16 for i/f/o)
KH = H // 128  # 8 h k-tiles (fp8 for i/f/o; bf16 for g)
NS = H // 128  # 8 h-slices of 128 hidden rows
GW = 4 * 128  # 512 bf16 M columns per h-slice (i|g|f|o)
G8W = 3 * 128  # 384 fp8 M columns per h-slice (i|f|o)
HALF = BLOC // 2  # 512-wide batch halves (one PSUM bank each)

_PROG = None


def _build_program():
    import concourse.mybir as mybir
    import concourse.tile as tile
    from concourse import bacc
    from contextlib import ExitStack

    f32 = mybir.dt.float32
    bf = mybir.dt.bfloat16
    f8 = mybir.dt.float8e4
    DR = mybir.MatmulPerfMode.DoubleRow
    SIG = mybir.ActivationFunctionType.Sigmoid
    TANH = mybir.ActivationFunctionType.Tanh

    nc = bacc.Bacc("TRN2", target_bir_lowering=False, debug=False)

    # a: full bf16 activations (x+h, for g and the x-part of i/f/o);
    # a8: fp8 copy of the h rows (for i/f/o). m: full bf16 weights;
    # m8: fp8 h-rows of the i/f/o columns, 384 per slice.
    a_d = nc.dram_tensor("a_in", [KTOT, BLOC], bf, kind="ExternalInput").ap()
    a8_d = nc.dram_tensor("a8_in", [H, BLOC], f8, kind="ExternalInput").ap()
    m_d = nc.dram_tensor("m_in", [KTOT, 4 * H], bf, kind="ExternalInput").ap()
    m8_d = nc.dram_tensor("m8_in", [H, 3 * H], f8, kind="ExternalInput").ap()
    bias_d = nc.dram_tensor("bias_in", [128, 4 * NS], f32, kind="ExternalInput").ap()
    bdi_d = nc.dram_tensor("bdi_in", [H, BLOC], f32, kind="ExternalInput").ap()
    ct_d = nc.dram_tensor("ct_in", [H, BLOC], f32, kind="ExternalInput").ap()
    ht_o = nc.dram_tensor("ht_out", [H, BLOC], f32, kind="ExternalOutput").ap()
    ct_o = nc.dram_tensor("ct_out", [H, BLOC], f32, kind="ExternalOutput").ap()

    with tile.TileContext(nc) as tc:
        with ExitStack() as ctx:
            apl = ctx.enter_context(tc.tile_pool(name="apl", bufs=1))
            mp = ctx.enter_context(tc.tile_pool(name="mp", bufs=3))
            cst = ctx.enter_context(tc.tile_pool(name="cst", bufs=1))
            ctp = ctx.enter_context(tc.tile_pool(name="ctp", bufs=2))
            gp = ctx.enter_context(tc.tile_pool(name="gp", bufs=6))
            ep = ctx.enter_context(tc.tile_pool(name="ep", bufs=4))
            outp = ctx.enter_context(tc.tile_pool(name="outp", bufs=4))
            psp = ctx.enter_context(tc.tile_pool(name="psp", bufs=8, space="PSUM"))
            wup = ctx.enter_context(tc.tile_pool(name="wup", bufs=1))

            wu_w = wup.tile([128, 128], bf, name="wu_w")
            nc.vector.memset(wu_w, 0.0)
            wu_ps = psp.tile([128, 512], f32, name="wu_ps", tag="ps")
            for _ in range(52):
                nc.tensor.matmul(wu_ps[:, 0:128], wu_w, wu_w, start=True, stop=True)

            bias_t = cst.tile([128, 4 * NS], f32, name="bias_t")
            nc.scalar.dma_start(out=bias_t, in_=bias_d[:, :])

            # A: fp8 h-rows in 4 chunks (sync), bf16 full in 6 chunks
            # (alternating sync/gpsimd for issue overlap)
            a8_ts = []
            for j in range(4):
                at = apl.tile([128, 2, BLOC], f8, name=f"a8_t{j}")
                nc.sync.dma_start(
                    out=at,
                    in_=a8_d[j * 256 : (j + 1) * 256, :].rearrange(
                        "(kk p) b -> p kk b", p=128
                    ),
                )
                a8_ts.append(at)
            a_ts = []
            for j in range(6):
                at = apl.tile([128, 2, BLOC], bf, name=f"a_t{j}")
                eng = nc.gpsimd if j % 2 == 0 else nc.sync
                eng.dma_start(
                    out=at,
                    in_=a_d[j * 256 : (j + 1) * 256, :].rearrange(
                        "(kk p) b -> p kk b", p=128
                    ),
                )
                a_ts.append(at)

            def a_ap(k, bs):
                return a_ts[k // 2][:, k % 2, bs]

            def a8_ap(kk8, bs):
                return a8_ts[kk8 // 2][:, :, bs]

            def load_m_slice(s):
                t8 = mp.tile([128, KH, G8W], f8, name=f"m8_{s}", tag="m8")
                for j in range(2):
                    nc.gpsimd.dma_start(
                        out=t8[:, j * 4 : (j + 1) * 4, :],
                        in_=m8_d[
                            j * 512 : (j + 1) * 512, s * G8W : (s + 1) * G8W
                        ].rearrange("(kk p) g -> p kk g", p=128),
                    )
                t = mp.tile([128, KT, GW], bf, name=f"m_{s}", tag="m")
                for j in range(3):
                    nc.sync.dma_start(
                        out=t[:, j * 4 : (j + 1) * 4, :],
                        in_=m_d[
                            j * 512 : (j + 1) * 512, s * GW : (s + 1) * GW
                        ].rearrange("(kk p) g -> p kk g", p=128),
                    )
                return t8, t

            def load_ct_slice(s):
                t = ctp.tile([128, BLOC], f32, name=f"ct_{s}", tag="ct")
                nc.scalar.dma_start(out=t, in_=ct_d[s * 128 : (s + 1) * 128, :])
                return t

            def load_bdi_slice(s):
                t = ctp.tile([128, BLOC], f32, name=f"bdi_{s}", tag="bdi")
                nc.scalar.dma_start(out=t, in_=bdi_d[s * 128 : (s + 1) * 128, :])
                return t

            def sig_gate_mms(ps, m8_t, m_t, g3, g4, bs):
                """i/f/o gate: 4 fp8 DR matmuls (h) + 4 bf16 matmuls (x)."""
                for kk8 in range(0, KH, 2):
                    nc.tensor.matmul(
                        ps, m8_t[:, kk8 : kk8 + 2, g3 * 128 : (g3 + 1) * 128],
                        a8_ap(kk8, bs), start=(kk8 == 0), stop=False,
                        perf_mode=DR,
                    )
                for k in range(KX):
                    nc.tensor.matmul(
                        ps, m_t[:, k, g4 * 128 : (g4 + 1) * 128], a_ap(k, bs),
                        start=False, stop=(k == KX - 1),
                    )

            def g_gate_mms(ps, m_t, bs):
                """g gate: all 12 k-tiles bf16."""
                for k in range(KT):
                    nc.tensor.matmul(
                        ps, m_t[:, k, 128:256], a_ap(k, bs),
                        start=(k == 0), stop=(k == KT - 1),
                    )

            def gate_acts(s, h2, ps_i, ps_g, ps_f, ps_o, ct_t, bdi_t):
                b0 = 4 * s
                bs = slice(h2 * HALF, (h2 + 1) * HALF)
                i_t = gp.tile([128, HALF], f32, name=f"i{s}_{h2}", tag="g")
                g_t = gp.tile([128, HALF], f32, name=f"g{s}_{h2}", tag="g")
                nc.scalar.activation(i_t, ps_i, SIG, bias=bias_t[:, b0 : b0 + 1])
                nc.scalar.activation(g_t, ps_g, TANH, bias=bias_t[:, b0 + 1 : b0 + 2])
                ig_t = ep.tile([128, HALF], f32, name=f"ig{s}_{h2}", tag="ig")
                nc.vector.tensor_mul(ig_t, i_t, g_t)
                f_t = gp.tile([128, HALF], f32, name=f"f{s}_{h2}", tag="g")
                o_t = gp.tile([128, HALF], f32, name=f"o{s}_{h2}", tag="g")

                hs = slice(s * 128, (s + 1) * 128)
                cn = outp.tile([128, HALF], f32, name=f"cn{s}_{h2}", tag="cn")
                th = ep.tile([128, HALF], f32, name=f"th{s}_{h2}", tag="th")
                hn = outp.tile([128, HALF], f32, name=f"hn{s}_{h2}", tag="hn")
                last = s == NS - 1 and h2 == 1
                for q0, q1 in ([(0, 256), (256, HALF)] if last else [(0, HALF)]):
                    qs = slice(q0, q1)
                    bqs = slice(h2 * HALF + q0, h2 * HALF + q1)
                    nc.vector.tensor_add(ps_f[:, qs], ps_f[:, qs], bdi_t[:, bqs])
                    nc.scalar.activation(
                        f_t[:, qs], ps_f[:, qs], SIG, bias=bias_t[:, b0 + 2 : b0 + 3]
                    )
                    nc.scalar.activation(
                        o_t[:, qs], ps_o[:, qs], SIG, bias=bias_t[:, b0 + 3 : b0 + 4]
                    )
                    nc.vector.tensor_mul(cn[:, qs], f_t[:, qs], ct_t[:, bqs])
                    nc.vector.tensor_add(cn[:, qs], cn[:, qs], ig_t[:, qs])
                    nc.scalar.activation(th[:, qs], cn[:, qs], TANH)
                    nc.vector.tensor_mul(hn[:, qs], o_t[:, qs], th[:, qs])
                nc.gpsimd.dma_start(out=ct_o[hs, bs], in_=cn)
                (nc.scalar if last else nc.gpsimd).dma_start(
                    out=ht_o[hs, bs], in_=hn
                )

            for s in range(NS):
                m8_t, m_t = load_m_slice(s)
                ct_t = load_ct_slice(s)
                bdi_t = load_bdi_slice(s)
                for h2 in range(2):
                    bs = slice(h2 * HALF, (h2 + 1) * HALF)
                    # wave 1: i (fp8 h + bf16 x), g (bf16 all)
                    ps_i = psp.tile([128, HALF], f32, name=f"psi{s}_{h2}", tag="ps")
                    ps_g = psp.tile([128, HALF], f32, name=f"psg{s}_{h2}", tag="ps")
                    sig_gate_mms(ps_i, m8_t, m_t, 0, 0, bs)
                    g_gate_mms(ps_g, m_t, bs)
                    # wave 2: f, o
                    ps_f = psp.tile([128, HALF], f32, name=f"psf{s}_{h2}", tag="ps")
                    ps_o = psp.tile([128, HALF], f32, name=f"pso{s}_{h2}", tag="ps")
                    sig_gate_mms(ps_f, m8_t, m_t, 1, 2, bs)
                    sig_gate_mms(ps_o, m8_t, m_t, 2, 3, bs)
                    gate_acts(s, h2, ps_i, ps_g, ps_f, ps_o, ct_t, bdi_t)
    nc.compile()
    return nc


def _get_program():
    global _PROG
    if _PROG is None:
        _PROG = _build_program()
    return _PROG


def _prep_inputs(inputs):
    f = np.float32
    x = np.asarray(inputs["x"], f)
    h_prev = np.asarray(inputs["h_prev"], f)
    c_prev = np.asarray(inputs["c_prev"], f)
    boundary = np.asarray(inputs["boundary"], f)

    gates = ["i", "g", "f", "o"]
    W = {z: np.asarray(inputs[f"W_{z}"], f) for z in gates}
    U = {z: np.asarray(inputs[f"U_{z}"], f) for z in gates}
    bias = {
        z: np.asarray(inputs[f"b_W{z}"], f) + np.asarray(inputs[f"b_U{z}"], f)
        for z in gates
    }
    W_b = np.asarray(inputs["W_b"], f)
    b_Wb = np.asarray(inputs["b_Wb"], f)
    bias["f"] = bias["f"] + b_Wb

    # bf16 M [1536, 4096]: per-slice columns [i | g | f | o] (for i/f/o only
    # the x rows are read on-device; g reads all rows).
    # fp8 M8 [1024, 3072]: h rows only, per-slice columns [i | f | o].
    M = np.empty((KTOT, 4 * H), f)
    M8 = np.empty((H, 3 * H), f)
    BIAS = np.empty((128, 4 * NS), f)
    sig_gates = ["i", "f", "o"]
    for s in range(NS):
        hs = slice(s * 128, (s + 1) * 128)
        for gi, z in enumerate(gates):
            cs = slice(s * GW + gi * 128, s * GW + (gi + 1) * 128)
            M[:IN, cs] = W[z][hs].T
            M[IN:, cs] = U[z][hs].T
            BIAS[:, 4 * s + gi] = bias[z][hs]
        for gi, z in enumerate(sig_gates):
            cs8 = slice(s * G8W + gi * 128, s * G8W + (gi + 1) * 128)
            M8[:, cs8] = U[z][hs].T

    Mb = M.astype(BF16)
    M8q = M8.astype(FP8)
    AT = np.concatenate([x, h_prev], axis=1).T  # [1536, 8192] f32
    ATb = np.ascontiguousarray(AT).astype(BF16)
    AT8 = np.ascontiguousarray(AT[IN:]).astype(FP8)  # h rows
    BDI = (boundary @ W_b.T).astype(f)

    in_maps = []
    for c in range(NCORES):
        rs = slice(c * BLOC, (c + 1) * BLOC)
        in_maps.append(
            {
                "a_in": np.ascontiguousarray(ATb[:, rs]),
                "a8_in": np.ascontiguousarray(AT8[:, rs]),
                "m_in": Mb,
                "m8_in": M8q,
                "bias_in": BIAS,
                "bdi_in": np.ascontiguousarray(BDI[rs].T),
                "ct_in": np.ascontiguousarray(c_prev[rs].T),
            }
        )
    return in_maps


def run(inputs, trace=False):
    from concourse.bass_utils import run_bass_kernel_spmd

    nc = _get_program()
    in_maps = _prep_inputs(inputs)
    res = run_bass_kernel_spmd(
        nc, in_maps, core_ids=list(range(NCORES)), trace=trace
    )
    h = np.concatenate(
        [np.ascontiguousarray(r["ht_out"].T) for r in res.results], axis=0
    )
    c = np.concatenate(
        [np.ascontiguousarray(r["ct_out"].T) for r in res.results], axis=0
    )
    return (h, c), res


def kernel(**inputs):
    out, _ = run(inputs, trace=False)
    return out


# revision 77
# speedup vs baseline: 1.0143x; 1.0143x over previous
"""Trainium2 Bass kernel for a custom LSTM cell.

Transposed layout (hidden on partitions, batch free), data-parallel over
batch across 8 cores. Gate biases ride the ACT engine's per-partition bias;
the boundary term is host-precomputed and DVE-added into the f-gate PSUM.

Precision: the g gate feeds tanh (derivative up to 1.0) and amplifies
quantization noise ~4x more than the sigmoid gates, so fp8 DoubleRow (2x PE
rate) is allocated per-gate: i/f/o take their ENTIRE h_prev contraction
(1024 rows) in fp8, g stays fully bf16. Measured end-to-end rel err
1.307e-2 vs the 2e-2 budget — lower than a uniform fp8 split, with 50%
more of the stream at 2x rate.
"""

import sys

sys.path.insert(0, "/opt/trn_rl_repo")

import numpy as np
import ml_dtypes

BF16 = ml_dtypes.bfloat16
FP8 = ml_dtypes.float8_e4m3  # matches mybir.dt.float8e4

B, IN, H = 8192, 512, 1024
NCORES = 8
BLOC = B // NCORES  # 1024 batch rows per core
KTOT = IN + H  # 1536 contraction
KT = KTOT // 12 * 12 // 128  # 12 k-tiles
KX = IN // 128  # 4 x k-tiles (bf16 for i/f/o)
KH = H // 128  # 8 h k-tiles (fp8 for i/f/o; bf16 for g)
NS = H // 128  # 8 h-slices of 128 hidden rows
GW = 4 * 128  # 512 bf16 M columns per h-slice (i|g|f|o)
G8W = 3 * 128  # 384 fp8 M columns per h-slice (i|f|o)
HALF = BLOC // 2  # 512-wide batch halves (one PSUM bank each)

_PROG = None


def _build_program():
    import concourse.mybir as mybir
    import concourse.tile as tile
    from concourse import bacc
    from contextlib import ExitStack

    f32 = mybir.dt.float32
    bf = mybir.dt.bfloat16
    f8 = mybir.dt.float8e4
    DR = mybir.MatmulPerfMode.DoubleRow
    SIG = mybir.ActivationFunctionType.Sigmoid
    TANH = mybir.ActivationFunctionType.Tanh

    nc = bacc.Bacc("TRN2", target_bir_lowering=False, debug=False)

    # a: full bf16 activations (x+h, for g and the x-part of i/f/o);
    # a8: fp8 copy of the h rows (for i/f/o). m: full bf16 weights;
    # m8: fp8 h-rows of the i/f/o columns, 384 per slice.
    a_d = nc.dram_tensor("a_in", [KTOT, BLOC], bf, kind="ExternalInput").ap()
    a8_d = nc.dram_tensor("a8_in", [H, BLOC], f8, kind="ExternalInput").ap()
    mx_d = nc.dram_tensor("mx_in", [IN, 3 * H], bf, kind="ExternalInput").ap()
    mg_d = nc.dram_tensor("mg_in", [KTOT, H], bf, kind="ExternalInput").ap()
    m8_d = nc.dram_tensor("m8_in", [H, 3 * H], f8, kind="ExternalInput").ap()
    bias_d = nc.dram_tensor("bias_in", [128, 4 * NS], f32, kind="ExternalInput").ap()
    bdi_d = nc.dram_tensor("bdi_in", [H, BLOC], f32, kind="ExternalInput").ap()
    ct_d = nc.dram_tensor("ct_in", [H, BLOC], f32, kind="ExternalInput").ap()
    ht_o = nc.dram_tensor("ht_out", [H, BLOC], f32, kind="ExternalOutput").ap()
    ct_o = nc.dram_tensor("ct_out", [H, BLOC], f32, kind="ExternalOutput").ap()

    with tile.TileContext(nc) as tc:
        with ExitStack() as ctx:
            apl = ctx.enter_context(tc.tile_pool(name="apl", bufs=1))
            mp = ctx.enter_context(tc.tile_pool(name="mp", bufs=3))
            cst = ctx.enter_context(tc.tile_pool(name="cst", bufs=1))
            ctp = ctx.enter_context(tc.tile_pool(name="ctp", bufs=2))
            gp = ctx.enter_context(tc.tile_pool(name="gp", bufs=6))
            ep = ctx.enter_context(tc.tile_pool(name="ep", bufs=4))
            outp = ctx.enter_context(tc.tile_pool(name="outp", bufs=4))
            psp = ctx.enter_context(tc.tile_pool(name="psp", bufs=8, space="PSUM"))
            wup = ctx.enter_context(tc.tile_pool(name="wup", bufs=1))

            wu_w = wup.tile([128, 128], bf, name="wu_w")
            nc.vector.memset(wu_w, 0.0)
            wu_ps = psp.tile([128, 512], f32, name="wu_ps", tag="ps")
            for _ in range(52):
                nc.tensor.matmul(wu_ps[:, 0:128], wu_w, wu_w, start=True, stop=True)

            bias_t = cst.tile([128, 4 * NS], f32, name="bias_t")
            nc.scalar.dma_start(out=bias_t, in_=bias_d[:, :])

            # A: fp8 h-rows in 4 chunks (sync), bf16 full in 6 chunks
            # (alternating sync/gpsimd for issue overlap)
            a8_ts = []
            for j in range(4):
                at = apl.tile([128, 2, BLOC], f8, name=f"a8_t{j}")
                nc.sync.dma_start(
                    out=at,
                    in_=a8_d[j * 256 : (j + 1) * 256, :].rearrange(
                        "(kk p) b -> p kk b", p=128
                    ),
                )
                a8_ts.append(at)
            a_ts = []
            for j in range(6):
                at = apl.tile([128, 2, BLOC], bf, name=f"a_t{j}")
                eng = nc.gpsimd if j % 2 == 0 else nc.sync
                eng.dma_start(
                    out=at,
                    in_=a_d[j * 256 : (j + 1) * 256, :].rearrange(
                        "(kk p) b -> p kk b", p=128
                    ),
                )
                a_ts.append(at)

            def a_ap(k, bs):
                return a_ts[k // 2][:, k % 2, bs]

            def a8_ap(kk8, bs):
                return a8_ts[kk8 // 2][:, :, bs]

            def load_m_slice(s):
                t8 = mp.tile([128, KH, G8W], f8, name=f"m8_{s}", tag="m8")
                for j in range(2):
                    nc.gpsimd.dma_start(
                        out=t8[:, j * 4 : (j + 1) * 4, :],
                        in_=m8_d[
                            j * 512 : (j + 1) * 512, s * G8W : (s + 1) * G8W
                        ].rearrange("(kk p) g -> p kk g", p=128),
                    )
                tx = mp.tile([128, KX, G8W], bf, name=f"mx_{s}", tag="mx")
                nc.sync.dma_start(
                    out=tx,
                    in_=mx_d[:, s * G8W : (s + 1) * G8W].rearrange(
                        "(kk p) g -> p kk g", p=128
                    ),
                )
                tg = mp.tile([128, KT, 128], bf, name=f"mg_{s}", tag="mg")
                for j in range(3):
                    nc.sync.dma_start(
                        out=tg[:, j * 4 : (j + 1) * 4, :],
                        in_=mg_d[
                            j * 512 : (j + 1) * 512, s * 128 : (s + 1) * 128
                        ].rearrange("(kk p) g -> p kk g", p=128),
                    )
                return t8, tx, tg

            def load_ct_slice(s):
                t = ctp.tile([128, BLOC], f32, name=f"ct_{s}", tag="ct")
                nc.scalar.dma_start(out=t, in_=ct_d[s * 128 : (s + 1) * 128, :])
                return t

            def load_bdi_slice(s):
                t = ctp.tile([128, BLOC], f32, name=f"bdi_{s}", tag="bdi")
                nc.scalar.dma_start(out=t, in_=bdi_d[s * 128 : (s + 1) * 128, :])
                return t

            def sig_gate_mms(ps, m8_t, m_t, g3, g4, bs):
                """i/f/o gate: 4 fp8 DR matmuls (h) + 4 bf16 matmuls (x)."""
                for kk8 in range(0, KH, 2):
                    nc.tensor.matmul(
                        ps, m8_t[:, kk8 : kk8 + 2, g3 * 128 : (g3 + 1) * 128],
                        a8_ap(kk8, bs), start=(kk8 == 0), stop=False,
                        perf_mode=DR,
                    )
                for k in range(KX):
                    nc.tensor.matmul(
                        ps, m_t[:, k, g4 * 128 : (g4 + 1) * 128], a_ap(k, bs),
                        start=False, stop=(k == KX - 1),
                    )

            def g_gate_mms(ps, m_t, bs):
                """g gate: all 12 k-tiles bf16."""
                for k in range(KT):
                    nc.tensor.matmul(
                        ps, m_t[:, k, :], a_ap(k, bs),
                        start=(k == 0), stop=(k == KT - 1),
                    )

            def gate_acts(s, h2, ps_i, ps_g, ps_f, ps_o, ct_t, bdi_t):
                b0 = 4 * s
                bs = slice(h2 * HALF, (h2 + 1) * HALF)
                i_t = gp.tile([128, HALF], f32, name=f"i{s}_{h2}", tag="g")
                g_t = gp.tile([128, HALF], f32, name=f"g{s}_{h2}", tag="g")
                f_t = gp.tile([128, HALF], f32, name=f"f{s}_{h2}", tag="g")
                o_t = gp.tile([128, HALF], f32, name=f"o{s}_{h2}", tag="g")
                ig_t = ep.tile([128, HALF], f32, name=f"ig{s}_{h2}", tag="ig")

                hs = slice(s * 128, (s + 1) * 128)
                cn = outp.tile([128, HALF], f32, name=f"cn{s}_{h2}", tag="cn")
                th = ep.tile([128, HALF], f32, name=f"th{s}_{h2}", tag="th")
                hn = outp.tile([128, HALF], f32, name=f"hn{s}_{h2}", tag="hn")
                last = s == NS - 1 and h2 == 1
                # i/f/o drain first (their PSUMs stop before g's); everything
                # g-dependent comes last so the DVE/ACT queues never block on
                # the g matmuls
                nc.scalar.activation(i_t, ps_i, SIG, bias=bias_t[:, b0 : b0 + 1])
                nc.vector.tensor_add(ps_f, ps_f, bdi_t[:, bs])
                nc.scalar.activation(f_t, ps_f, SIG, bias=bias_t[:, b0 + 2 : b0 + 3])
                nc.scalar.activation(o_t, ps_o, SIG, bias=bias_t[:, b0 + 3 : b0 + 4])
                nc.vector.tensor_mul(cn, f_t, ct_t[:, bs])
                for q0, q1 in ([(0, 256), (256, HALF)] if last else [(0, HALF)]):
                    qs = slice(q0, q1)
                    nc.scalar.activation(
                        g_t[:, qs], ps_g[:, qs], TANH,
                        bias=bias_t[:, b0 + 1 : b0 + 2],
                    )
                    nc.vector.tensor_mul(ig_t[:, qs], i_t[:, qs], g_t[:, qs])
                    nc.vector.tensor_add(cn[:, qs], cn[:, qs], ig_t[:, qs])
                    nc.scalar.activation(th[:, qs], cn[:, qs], TANH)
                    nc.vector.tensor_mul(hn[:, qs], o_t[:, qs], th[:, qs])
                nc.gpsimd.dma_start(out=ct_o[hs, bs], in_=cn)
                (nc.scalar if last else nc.gpsimd).dma_start(
                    out=ht_o[hs, bs], in_=hn
                )

            for s in range(NS):
                m8_t, mx_t, mg_t = load_m_slice(s)
                ct_t = load_ct_slice(s)
                bdi_t = load_bdi_slice(s)
                for h2 in range(2):
                    bs = slice(h2 * HALF, (h2 + 1) * HALF)
                    ps_i = psp.tile([128, HALF], f32, name=f"psi{s}_{h2}", tag="ps")
                    ps_f = psp.tile([128, HALF], f32, name=f"psf{s}_{h2}", tag="ps")
                    ps_o = psp.tile([128, HALF], f32, name=f"pso{s}_{h2}", tag="ps")
                    ps_g = psp.tile([128, HALF], f32, name=f"psg{s}_{h2}", tag="ps")
                    # all fp8 DR matmuls first: they depend only on the small
                    # m8/a8 tensors, which land earliest
                    for g3, ps in ((0, ps_i), (1, ps_f), (2, ps_o)):
                        for kk8 in range(0, KH, 2):
                            nc.tensor.matmul(
                                ps,
                                m8_t[:, kk8 : kk8 + 2, g3 * 128 : (g3 + 1) * 128],
                                a8_ap(kk8, bs), start=(kk8 == 0), stop=False,
                                perf_mode=DR,
                            )
                    # then the bf16 x-parts
                    for g3, ps in ((0, ps_i), (1, ps_f), (2, ps_o)):
                        for k in range(KX):
                            nc.tensor.matmul(
                                ps, mx_t[:, k, g3 * 128 : (g3 + 1) * 128],
                                a_ap(k, bs), start=False, stop=(k == KX - 1),
                            )
                    # g gate last: it needs the full bf16 weight tile and all
                    # of A, the slowest-arriving data
                    g_gate_mms(ps_g, mg_t, bs)
                    gate_acts(s, h2, ps_i, ps_g, ps_f, ps_o, ct_t, bdi_t)
    nc.compile()
    return nc


def _get_program():
    global _PROG
    if _PROG is None:
        _PROG = _build_program()
    return _PROG


def _prep_inputs(inputs):
    f = np.float32
    x = np.asarray(inputs["x"], f)
    h_prev = np.asarray(inputs["h_prev"], f)
    c_prev = np.asarray(inputs["c_prev"], f)
    boundary = np.asarray(inputs["boundary"], f)

    gates = ["i", "g", "f", "o"]
    W = {z: np.asarray(inputs[f"W_{z}"], f) for z in gates}
    U = {z: np.asarray(inputs[f"U_{z}"], f) for z in gates}
    bias = {
        z: np.asarray(inputs[f"b_W{z}"], f) + np.asarray(inputs[f"b_U{z}"], f)
        for z in gates
    }
    W_b = np.asarray(inputs["W_b"], f)
    b_Wb = np.asarray(inputs["b_Wb"], f)
    bias["f"] = bias["f"] + b_Wb

    # bf16 M [1536, 4096]: per-slice columns [i | g | f | o] (for i/f/o only
    # the x rows are read on-device; g reads all rows).
    # fp8 M8 [1024, 3072]: h rows only, per-slice columns [i | f | o].
    MX = np.empty((IN, 3 * H), f)
    MG = np.empty((KTOT, H), f)
    M8 = np.empty((H, 3 * H), f)
    BIAS = np.empty((128, 4 * NS), f)
    sig_gates = ["i", "f", "o"]
    for s in range(NS):
        hs = slice(s * 128, (s + 1) * 128)
        for gi, z in enumerate(gates):
            BIAS[:, 4 * s + gi] = bias[z][hs]
        for gi, z in enumerate(sig_gates):
            cs8 = slice(s * G8W + gi * 128, s * G8W + (gi + 1) * 128)
            MX[:, cs8] = W[z][hs].T
            M8[:, cs8] = U[z][hs].T
        MG[:IN, s * 128 : (s + 1) * 128] = W["g"][hs].T
        MG[IN:, s * 128 : (s + 1) * 128] = U["g"][hs].T

    MXb = MX.astype(BF16)
    MGb = MG.astype(BF16)
    M8q = M8.astype(FP8)
    AT = np.concatenate([x, h_prev], axis=1).T  # [1536, 8192] f32
    ATb = np.ascontiguousarray(AT).astype(BF16)
    AT8 = np.ascontiguousarray(AT[IN:]).astype(FP8)  # h rows
    BDI = (boundary @ W_b.T).astype(f)

    in_maps = []
    for c in range(NCORES):
        rs = slice(c * BLOC, (c + 1) * BLOC)
        in_maps.append(
            {
                "a_in": np.ascontiguousarray(ATb[:, rs]),
                "a8_in": np.ascontiguousarray(AT8[:, rs]),
                "mx_in": MXb,
                "mg_in": MGb,
                "m8_in": M8q,
                "bias_in": BIAS,
                "bdi_in": np.ascontiguousarray(BDI[rs].T),
                "ct_in": np.ascontiguousarray(c_prev[rs].T),
            }
        )
    return in_maps


def run(inputs, trace=False):
    from concourse.bass_utils import run_bass_kernel_spmd

    nc = _get_program()
    in_maps = _prep_inputs(inputs)
    res = run_bass_kernel_spmd(
        nc, in_maps, core_ids=list(range(NCORES)), trace=trace
    )
    h = np.concatenate(
        [np.ascontiguousarray(r["ht_out"].T) for r in res.results], axis=0
    )
    c = np.concatenate(
        [np.ascontiguousarray(r["ct_out"].T) for r in res.results], axis=0
    )
    return (h, c), res


def kernel(**inputs):
    out, _ = run(inputs, trace=False)
    return out


# revision 78
# speedup vs baseline: 1.0169x; 1.0026x over previous
"""Trainium2 Bass kernel for a custom LSTM cell.

Math (per reference):
    i = sigmoid(x @ W_i.T + b_Wi + h @ U_i.T + b_Ui)
    f = sigmoid(x @ W_f.T + b_Wf + h @ U_f.T + b_Uf + boundary @ W_b.T + b_Wb)
    o = sigmoid(x @ W_o.T + b_Wo + h @ U_o.T + b_Uo)
    g = tanh   (x @ W_g.T + b_Wg + h @ U_g.T + b_Ug)
    c = f * c_prev + i * g
    h = o * tanh(c)

Strategy: data-parallel over batch across 8 NeuronCores (1024 rows each),
computed TRANSPOSED on-device: hidden on partitions, batch on the free axis.
With hidden on partitions the gate biases become per-partition ACT-engine
bias operands (free), and the boundary term (precomputed host-side as
boundary @ W_b.T) is added into the f-gate PSUM with one DVE op — the PE
stream is pure gate matmuls.

Matmul operands are bf16 (well within the 2e-2 error budget), halving HBM
traffic vs f32/f32r. Per h-slice of 128 hidden rows the gates run in two
waves (i,g then f,o) of [128,512] PSUM tiles so the 8 PSUM banks hold two
(slice, batch-half) units in flight and the PE never waits on drains.
Slice 0 is supply-limited (A + its weights stream in during the first
~14us), so it runs all 8 accumulators in one k-major pass whose chunked
DMA dependencies match the delivery order.
"""

import sys

sys.path.insert(0, "/opt/trn_rl_repo")

import numpy as np
import ml_dtypes

BF16 = ml_dtypes.bfloat16
FP8 = ml_dtypes.float8_e4m3  # matches mybir.dt.float8e4

B, IN, H = 8192, 512, 1024
NCORES = 8
BLOC = B // NCORES  # 1024 batch rows per core
KTOT = IN + H  # 1536 contraction
KT = KTOT // 128  # 12 k-tiles
KF8 = 4  # k-tiles computed in fp8 DoubleRow (2x PE rate), taken from the
# h-part (rows h 0:512), whose per-row quantization noise is half the
# x-part's; end-to-end rel err measured 1.48e-2 vs the 2e-2 budget
KB = KTOT - KF8 * 128  # 1024 bf16 contraction rows
NS = H // 128  # 8 h-slices of 128 hidden rows
GW = 4 * 128  # 512 columns of M per h-slice (i|g|f|o)
HALF = BLOC // 2  # 512-wide batch halves (one PSUM bank each)

_PROG = None  # cached so repeat calls skip rebuild/recompile


def _build_program():
    import concourse.mybir as mybir
    import concourse.tile as tile
    from concourse import bacc
    from contextlib import ExitStack

    f32 = mybir.dt.float32
    bf = mybir.dt.bfloat16
    f8 = mybir.dt.float8e4
    DR = mybir.MatmulPerfMode.DoubleRow
    SIG = mybir.ActivationFunctionType.Sigmoid
    TANH = mybir.ActivationFunctionType.Tanh

    nc = bacc.Bacc("TRN2", target_bir_lowering=False, debug=False)

    a8_d = nc.dram_tensor("a8_in", [KF8 * 128, BLOC], f8, kind="ExternalInput").ap()
    a_d = nc.dram_tensor("a_in", [KB, BLOC], bf, kind="ExternalInput").ap()
    m8_d = nc.dram_tensor("m8_in", [KF8 * 128, 4 * H], f8, kind="ExternalInput").ap()
    m_d = nc.dram_tensor("m_in", [KB, 4 * H], bf, kind="ExternalInput").ap()
    bias_d = nc.dram_tensor("bias_in", [128, 4 * NS], f32, kind="ExternalInput").ap()
    bdi_d = nc.dram_tensor("bdi_in", [H, BLOC], f32, kind="ExternalInput").ap()
    ct_d = nc.dram_tensor("ct_in", [H, BLOC], f32, kind="ExternalInput").ap()
    ht_o = nc.dram_tensor("ht_out", [H, BLOC], f32, kind="ExternalOutput").ap()
    ct_o = nc.dram_tensor("ct_out", [H, BLOC], f32, kind="ExternalOutput").ap()

    with tile.TileContext(nc) as tc:
        with ExitStack() as ctx:
            apl = ctx.enter_context(tc.tile_pool(name="apl", bufs=1))
            mp = ctx.enter_context(tc.tile_pool(name="mp", bufs=3))
            cst = ctx.enter_context(tc.tile_pool(name="cst", bufs=1))
            ctp = ctx.enter_context(tc.tile_pool(name="ctp", bufs=2))
            gp = ctx.enter_context(tc.tile_pool(name="gp", bufs=6))
            ep = ctx.enter_context(tc.tile_pool(name="ep", bufs=4))
            outp = ctx.enter_context(tc.tile_pool(name="outp", bufs=4))
            psp = ctx.enter_context(tc.tile_pool(name="psp", bufs=8, space="PSUM"))
            wup = ctx.enter_context(tc.tile_pool(name="wup", bufs=1))

            # Small PE warm-up: absorbs the p-state ramp while the first
            # activation/weight chunks land.
            wu_w = wup.tile([128, 128], bf, name="wu_w")
            nc.vector.memset(wu_w, 0.0)
            wu_ps = psp.tile([128, 512], f32, name="wu_ps", tag="ps")
            for _ in range(40):
                nc.tensor.matmul(wu_ps[:, 0:128], wu_w, wu_w, start=True, stop=True)

            bias_t = cst.tile([128, 4 * NS], f32, name="bias_t")
            nc.scalar.dma_start(out=bias_t, in_=bias_d[:, :])

            def load_m_slice(s):
                """fp8 [128, 2, 512] + bf16 [128, 10, 512] weight tiles for
                h-slice s."""
                t8 = mp.tile([128, KF8, GW], f8, name=f"m8_{s}", tag="m8")
                nc.sync.dma_start(
                    out=t8,
                    in_=m8_d[:, s * GW : (s + 1) * GW].rearrange(
                        "(kk p) g -> p kk g", p=128
                    ),
                )
                t = mp.tile([128, KT - KF8, GW], bf, name=f"m_{s}", tag="m")
                for j in range(2):
                    nc.sync.dma_start(
                        out=t[:, j * 4 : (j + 1) * 4, :],
                        in_=m_d[
                            j * 512 : (j + 1) * 512, s * GW : (s + 1) * GW
                        ].rearrange("(kk p) g -> p kk g", p=128),
                    )
                return t8, t

            def load_ct_slice(s, eng=None):
                t = ctp.tile([128, BLOC], f32, name=f"ct_{s}", tag="ct")
                (eng or nc.scalar).dma_start(
                    out=t, in_=ct_d[s * 128 : (s + 1) * 128, :]
                )
                return t

            def load_bdi_slice(s, eng=None):
                t = ctp.tile([128, BLOC], f32, name=f"bdi_{s}", tag="bdi")
                (eng or nc.scalar).dma_start(
                    out=t, in_=bdi_d[s * 128 : (s + 1) * 128, :]
                )
                return t

            # A and slice-0 weights land as separate kk=2 chunk tiles so each
            # matmul pair only waits on its own chunk, not the whole slice.
            # The fp8 pair goes first (tiny, feeds the opening DoubleRow
            # matmul); A issues on the sync queue, slice-0 weights on the
            # (otherwise idle at startup) gpsimd queue so the ~0.7us
            # per-issue costs overlap.
            a8_ts = []
            for j in range(KF8 // 2):
                at = apl.tile([128, 2, BLOC], f8, name=f"a8_t{j}")
                nc.sync.dma_start(
                    out=at,
                    in_=a8_d[j * 256 : (j + 1) * 256, :].rearrange(
                        "(kk p) b -> p kk b", p=128
                    ),
                )
                a8_ts.append(at)
            a_ts = []
            for j in range(4):
                at = apl.tile([128, 2, BLOC], bf, name=f"a_t{j}")
                nc.sync.dma_start(
                    out=at,
                    in_=a_d[j * 256 : (j + 1) * 256, :].rearrange(
                        "(kk p) b -> p kk b", p=128
                    ),
                )
                a_ts.append(at)
            m80_ts = []
            for j in range(KF8 // 2):
                mt = apl.tile([128, 2, GW], f8, name=f"m80_t{j}")
                nc.gpsimd.dma_start(
                    out=mt,
                    in_=m8_d[j * 256 : (j + 1) * 256, 0:GW].rearrange(
                        "(kk p) g -> p kk g", p=128
                    ),
                )
                m80_ts.append(mt)
            m0_ts = []
            for j in range(4):
                mt = apl.tile([128, 2, GW], bf, name=f"m0_t{j}")
                nc.gpsimd.dma_start(
                    out=mt,
                    in_=m_d[j * 256 : (j + 1) * 256, 0:GW].rearrange(
                        "(kk p) g -> p kk g", p=128
                    ),
                )
                m0_ts.append(mt)
            # slice-0 c_prev rides the scalar queue (small), boundary behind
            # the A chunks on sync: both are only needed at the slice-0 drain.
            ct_t = load_ct_slice(0)
            bdi_t = load_bdi_slice(0, eng=nc.sync)

            def a_ap(k, bs):
                """bf16 A chunk access for k-tiles KF8..KT-1."""
                return a_ts[(k - KF8) // 2][:, (k - KF8) % 2, bs]

            def gate_acts(s, h2, ps_i, ps_g, ps_f, ps_o, ct_t, bdi_t):
                """Activations + elementwise + stores for one (s, h2) unit."""
                b0 = 4 * s
                bs = slice(h2 * HALF, (h2 + 1) * HALF)
                i_t = gp.tile([128, HALF], f32, name=f"i{s}_{h2}", tag="g")
                g_t = gp.tile([128, HALF], f32, name=f"g{s}_{h2}", tag="g")
                nc.scalar.activation(i_t, ps_i, SIG, bias=bias_t[:, b0 : b0 + 1])
                nc.scalar.activation(g_t, ps_g, TANH, bias=bias_t[:, b0 + 1 : b0 + 2])
                ig_t = ep.tile([128, HALF], f32, name=f"ig{s}_{h2}", tag="ig")
                nc.vector.tensor_mul(ig_t, i_t, g_t)
                f_t = gp.tile([128, HALF], f32, name=f"f{s}_{h2}", tag="g")
                o_t = gp.tile([128, HALF], f32, name=f"o{s}_{h2}", tag="g")

                # c' = f*c_prev + i*g ; h = o*tanh(c'). The very last unit
                # runs in 256-wide chunks to shorten the serial tail chain.
                hs = slice(s * 128, (s + 1) * 128)
                cn = outp.tile([128, HALF], f32, name=f"cn{s}_{h2}", tag="cn")
                th = ep.tile([128, HALF], f32, name=f"th{s}_{h2}", tag="th")
                hn = outp.tile([128, HALF], f32, name=f"hn{s}_{h2}", tag="hn")
                last = s == NS - 1 and h2 == 1
                for q0, q1 in ([(0, 256), (256, HALF)] if last else [(0, HALF)]):
                    qs = slice(q0, q1)
                    bqs = slice(h2 * HALF + q0, h2 * HALF + q1)
                    # boundary influence lands in the f-gate PSUM via one DVE
                    # add (saves a K=2 matmul in the PE stream per unit)
                    nc.vector.tensor_add(
                        ps_f[:, qs], ps_f[:, qs], bdi_t[:, bqs]
                    )
                    nc.scalar.activation(
                        f_t[:, qs], ps_f[:, qs], SIG, bias=bias_t[:, b0 + 2 : b0 + 3]
                    )
                    nc.scalar.activation(
                        o_t[:, qs], ps_o[:, qs], SIG, bias=bias_t[:, b0 + 3 : b0 + 4]
                    )
                    nc.vector.tensor_mul(cn[:, qs], f_t[:, qs], ct_t[:, bqs])
                    nc.vector.tensor_add(cn[:, qs], cn[:, qs], ig_t[:, qs])
                    nc.scalar.activation(th[:, qs], cn[:, qs], TANH)
                    nc.vector.tensor_mul(hn[:, qs], o_t[:, qs], th[:, qs])
                nc.gpsimd.dma_start(out=ct_o[hs, bs], in_=cn)
                # the final h store issues on the scalar queue so its ~0.65us
                # issue cost overlaps the c store's instead of serializing
                (nc.scalar if last else nc.gpsimd).dma_start(
                    out=ht_o[hs, bs], in_=hn
                )

            # Slice 0 is supply-limited (A + its weights stream in during the
            # first ~14us): run both batch halves' 8 accumulators in one pass,
            # h2 interleaved inside k, so PE consumption per chunk stays
            # behind the DMA supply.
            ps0 = {}
            for h2 in range(2):
                for z in "igfo":
                    ps0[z, h2] = psp.tile(
                        [128, HALF], f32, name=f"ps{z}0_{h2}", tag="ps"
                    )

            def m0_ap(k, c0, c1):
                return m0_ts[(k - KF8) // 2][:, (k - KF8) % 2, c0:c1]

            # fp8 DoubleRow opener: k-tiles 0..KF8-1 in 2x-rate matmuls
            for kk8 in range(0, KF8, 2):
                for h2 in range(2):
                    bs = slice(h2 * HALF, (h2 + 1) * HALF)
                    rhs8 = a8_ts[kk8 // 2][:, :, bs]
                    for gi, z in enumerate("igfo"):
                        nc.tensor.matmul(
                            ps0[z, h2],
                            m80_ts[kk8 // 2][:, :, gi * 128 : (gi + 1) * 128],
                            rhs8, start=(kk8 == 0), stop=False, perf_mode=DR,
                        )
            for k in range(KF8, KT):
                sp = k == KT - 1
                for h2 in range(2):
                    bs = slice(h2 * HALF, (h2 + 1) * HALF)
                    rhs = a_ap(k, bs)
                    nc.tensor.matmul(
                        ps0["i", h2], m0_ap(k, 0, 128), rhs, start=False, stop=sp
                    )
                    nc.tensor.matmul(
                        ps0["g", h2], m0_ap(k, 128, 256), rhs, start=False, stop=sp
                    )
                    nc.tensor.matmul(
                        ps0["f", h2], m0_ap(k, 256, 384), rhs, start=False, stop=sp
                    )
                    nc.tensor.matmul(
                        ps0["o", h2], m0_ap(k, 384, 512), rhs, start=False, stop=sp
                    )
                # pace-matching pad: slice-0 consumption slightly outruns the
                # DMA supply; a zero-weight accumulate (adds exactly 0) keeps
                # the PE continuously busy so its p-state never resets.
                if k % 2 == 1 and k < KT - 1:
                    for _ in range(6):
                        nc.tensor.matmul(
                            ps0["i", 1][:, 0:128], wu_w, wu_w,
                            start=False, stop=False,
                        )
            for h2 in range(2):
                gate_acts(
                    0, h2, ps0["i", h2], ps0["g", h2], ps0["f", h2], ps0["o", h2],
                    ct_t, bdi_t,
                )

            for s in range(1, NS):
                m8_t, m_t = load_m_slice(s)
                ct_t = load_ct_slice(s)
                bdi_t = load_bdi_slice(s)
                for h2 in range(2):
                    bs = slice(h2 * HALF, (h2 + 1) * HALF)
                    # wave 1: i, g
                    ps_i = psp.tile([128, HALF], f32, name=f"psi{s}_{h2}", tag="ps")
                    ps_g = psp.tile([128, HALF], f32, name=f"psg{s}_{h2}", tag="ps")
                    for kk8 in range(0, KF8, 2):
                        k8 = slice(kk8, kk8 + 2)
                        rhs8 = a8_ts[kk8 // 2][:, :, bs]
                        nc.tensor.matmul(
                            ps_i, m8_t[:, k8, 0:128], rhs8,
                            start=(kk8 == 0), stop=False, perf_mode=DR,
                        )
                        nc.tensor.matmul(
                            ps_g, m8_t[:, k8, 128:256], rhs8,
                            start=(kk8 == 0), stop=False, perf_mode=DR,
                        )
                    for k in range(KF8, KT):
                        rhs = a_ap(k, bs)
                        nc.tensor.matmul(
                            ps_i, m_t[:, k - KF8, 0:128], rhs,
                            start=False, stop=(k == KT - 1),
                        )
                        nc.tensor.matmul(
                            ps_g, m_t[:, k - KF8, 128:256], rhs,
                            start=False, stop=(k == KT - 1),
                        )
                    # wave 2: f, o
                    ps_f = psp.tile([128, HALF], f32, name=f"psf{s}_{h2}", tag="ps")
                    ps_o = psp.tile([128, HALF], f32, name=f"pso{s}_{h2}", tag="ps")
                    for kk8 in range(0, KF8, 2):
                        k8 = slice(kk8, kk8 + 2)
                        rhs8 = a8_ts[kk8 // 2][:, :, bs]
                        nc.tensor.matmul(
                            ps_f, m8_t[:, k8, 256:384], rhs8,
                            start=(kk8 == 0), stop=False, perf_mode=DR,
                        )
                        nc.tensor.matmul(
                            ps_o, m8_t[:, k8, 384:512], rhs8,
                            start=(kk8 == 0), stop=False, perf_mode=DR,
                        )
                    for k in range(KF8, KT):
                        rhs = a_ap(k, bs)
                        nc.tensor.matmul(
                            ps_f, m_t[:, k - KF8, 256:384], rhs,
                            start=False, stop=(k == KT - 1),
                        )
                        nc.tensor.matmul(
                            ps_o, m_t[:, k - KF8, 384:512], rhs,
                            start=False, stop=(k == KT - 1),
                        )
                    gate_acts(s, h2, ps_i, ps_g, ps_f, ps_o, ct_t, bdi_t)
    nc.compile()
    return nc


def _get_program():
    global _PROG
    if _PROG is None:
        _PROG = _build_program()
    return _PROG


def _prep_inputs(inputs):
    """Host-side marshalling: fused bf16 weight matrix + transposed acts."""
    f = np.float32
    x = np.asarray(inputs["x"], f)
    h_prev = np.asarray(inputs["h_prev"], f)
    c_prev = np.asarray(inputs["c_prev"], f)
    boundary = np.asarray(inputs["boundary"], f)

    gates = ["i", "g", "f", "o"]
    W = {z: np.asarray(inputs[f"W_{z}"], f) for z in gates}
    U = {z: np.asarray(inputs[f"U_{z}"], f) for z in gates}
    bias = {
        z: np.asarray(inputs[f"b_W{z}"], f) + np.asarray(inputs[f"b_U{z}"], f)
        for z in gates
    }
    W_b = np.asarray(inputs["W_b"], f)
    b_Wb = np.asarray(inputs["b_Wb"], f)
    bias["f"] = bias["f"] + b_Wb

    # M [1536, 4096]: rows 0-511 W.T, rows 512-1535 U.T; columns grouped per
    # 128-wide h-slice as [i | g | f | o].
    M = np.empty((KTOT, 4 * H), f)
    BIAS = np.empty((128, 4 * NS), f)
    for s in range(NS):
        hs = slice(s * 128, (s + 1) * 128)
        for gi, z in enumerate(gates):
            cs = slice(s * GW + gi * 128, s * GW + (gi + 1) * 128)
            M[:IN, cs] = W[z][hs].T
            M[IN:, cs] = U[z][hs].T
            BIAS[:, 4 * s + gi] = bias[z][hs]

    # contraction rows are reordered: the fp8 block is h_prev rows 0:512
    # (half the per-row quantization noise of x rows), the bf16 block is
    # all of x plus h_prev rows 512:1024. The same permutation applies to
    # A and M, so the matmul result is unchanged.
    NF8 = KF8 * 128
    f8_rows = slice(IN, IN + NF8)
    M8 = np.ascontiguousarray(M[f8_rows]).astype(FP8)
    Mb = np.ascontiguousarray(
        np.concatenate([M[:IN], M[IN + NF8 :]], axis=0)
    ).astype(BF16)
    AT = np.concatenate([x, h_prev], axis=1).T  # [1536, 8192] f32
    AT8 = np.ascontiguousarray(AT[f8_rows]).astype(FP8)
    ATb = np.ascontiguousarray(
        np.concatenate([AT[:IN], AT[IN + NF8 :]], axis=0)
    ).astype(BF16)
    # boundary influence (minus its bias, already folded into BIAS) computed
    # host-side: [B, H] -> transposed per-core slices like c_prev
    BDI = (boundary @ W_b.T).astype(f)  # [8192, 1024]

    in_maps = []
    for c in range(NCORES):
        rs = slice(c * BLOC, (c + 1) * BLOC)
        in_maps.append(
            {
                "a8_in": np.ascontiguousarray(AT8[:, rs]),
                "a_in": np.ascontiguousarray(ATb[:, rs]),
                "m8_in": M8,
                "m_in": Mb,
                "bias_in": BIAS,
                "bdi_in": np.ascontiguousarray(BDI[rs].T),
                "ct_in": np.ascontiguousarray(c_prev[rs].T),
            }
        )
    return in_maps


def run(inputs, trace=False):
    """Returns ((h, c), BassKernelResults)."""
    from concourse.bass_utils import run_bass_kernel_spmd

    nc = _get_program()
    in_maps = _prep_inputs(inputs)
    res = run_bass_kernel_spmd(
        nc, in_maps, core_ids=list(range(NCORES)), trace=trace
    )
    h = np.concatenate(
        [np.ascontiguousarray(r["ht_out"].T) for r in res.results], axis=0
    )
    c = np.concatenate(
        [np.ascontiguousarray(r["ct_out"].T) for r in res.results], axis=0
    )
    return (h, c), res


def kernel(**inputs):
    out, _ = run(inputs, trace=False)
    return out
